# revision 1
# baseline (speedup 1.0000x reference)
"""CABlock (cross-attention block) Trainium2 Bass kernel.

Problem: b=8, c=64, h=w=48 (n=2304), CR=8.
  qk_i = Wqk_i @ x_i + bqk_i  (q = first 8 rows, k = last 8)
  attn_i = softmax_j(q_i^T k_i)            [n, n]
  o1 = (Wv1@x1 + bv1) @ attn2 * gamma + x1
  o2 = (Wv2@x2 + bv2) @ attn1 * beta  + x2

Sharding: data-parallel over batch, 1 batch element per NeuronCore (8 cores).

Per-core dataflow (channel-on-partition):
  - x packed [128, 2304] fp32r: x1 @ partitions 0:64, x2 @ 64:128.
  - q/k [128, 2*2304] fp32r at partitions 0:8 (attn1 cols 0:N, attn2 cols
    N:2N).  Logit matmuls (K=8) run in fp32r (full PE rate for N>=256, vs
    4x slower plain fp32); fp32r requires dst psum partition base 0.
  - A-row (128 queries x 2304 keys) computed in 512-col PSUM chunks through
    3 rotating single-bank tiles; ScalarE exp's each chunk PSUM->SBUF(bf16)
    with accum_out giving softmax row-sums for free.  No max subtraction
    (logit range is far inside fp32 exp range).
  - 1/s (and gamma/beta) folded into the tiny [128, 64] V^T tiles instead of
    the n x n matrix.  E and V^T in bf16 -> o-matmuls run at full rate and
    may use column tiling (o2 at psum partitions 64:128); the attention term
    is scaled by gamma/beta = 0.1, so bf16 rounding there is ~5e-4 relative
    to the residual-dominated output.
  - o1/o2 column-packed into one PSUM accumulator [128, 2304] (5 banks),
    accumulated over all 18 i-tiles via start/stop; final residual add on DVE.
"""

import numpy as np

C = 64
CR = 8
H = W = 48
N = H * W            # 2304
B = 8
P = 128
IT = N // P          # 18 i-tiles
CHUNKS = [(0, 512), (512, 512), (1024, 512), (1536, 512), (2048, 256)]
NCHUNK = len(CHUNKS)

_CACHE = {}


def _build(repeats=1):
    import concourse.bacc as bacc
    import concourse.tile as tile
    from concourse import mybir

    F32 = mybir.dt.float32
    F32R = mybir.dt.float32r
    BF16 = mybir.dt.bfloat16
    AF = mybir.ActivationFunctionType
    ALU = mybir.AluOpType
    AX = mybir.AxisListType

    nc = bacc.Bacc("TRN2", target_bir_lowering=False, debug=False, num_devices=8)

    x1_d = nc.dram_tensor("x1", [C, N], F32R, kind="ExternalInput")
    x2_d = nc.dram_tensor("x2", [C, N], F32R, kind="ExternalInput")
    # consts columns: 0:8 wqT, 8:16 wkT, 16:80 wvT, 80 q1bias, 81 k1bias,
    # 82 q2bias, 83 k2bias (rows 0:8), 84:148 bv1 bcast, 148:212 bv2 bcast,
    # 212 gamma, 213 beta
    cst_d = nc.dram_tensor("consts", [P, 214], F32R, kind="ExternalInput")
    out_d = nc.dram_tensor("out", [P, N], F32, kind="ExternalOutput")

    with tile.TileContext(nc) as tc:
        with (
            tc.tile_pool(name="big", bufs=1) as big,
            tc.tile_pool(name="epool", bufs=6) as epool,
            tc.tile_pool(name="small", bufs=4) as small,
            tc.tile_pool(name="psA", bufs=3, space="PSUM") as psA,
            tc.tile_pool(name="psO", bufs=1, space="PSUM") as psO,
        ):
            # ---- early ACT table warm (loads exp tables during DMA wait)
            warm = big.tile([P, 1], F32, name="warm", tag="warm")
            warm2 = big.tile([P, 1], F32, name="warm2", tag="warm2")
            nc.vector.memset(warm, 0.0)
            nc.scalar.activation(out=warm2, in_=warm, func=AF.Exp)

            # ---- constant + input DMAs
            cst = big.tile([P, 214], F32R, name="cst", tag="cst")
            nc.sync.dma_start(out=cst, in_=cst_d.ap())
            x_sb = big.tile([P, N], F32R, name="x_sb", tag="x_sb")
            nc.sync.dma_start(out=x_sb[0:C, :], in_=x1_d.ap())
            nc.sync.dma_start(out=x_sb[C:P, :], in_=x2_d.ap())

            wq = cst[:, 0:8]
            wk = cst[:, 8:16]
            wv = cst[:, 16:80]
            qkbias = [[cst[:, 80:81].bitcast(F32), cst[:, 81:82].bitcast(F32)],
                      [cst[:, 82:83].bitcast(F32), cst[:, 83:84].bitcast(F32)]]
            bvb = [cst[:, 84:148].bitcast(F32), cst[:, 148:212].bitcast(F32)]
            gamma = cst[:, 212:213].bitcast(F32)
            beta = cst[:, 213:214].bitcast(F32)

            # ---- PE HAM warm-up: ~3.4us of dummy matmuls during DMA wait
            wz = big.tile([P, 512], BF16, name="wz", tag="wz")
            nc.vector.memset(wz, 0.0)
            wps = psA.tile([P, 512], F32, name="wps", tag="ps")
            for _wi in range(16):
                nc.tensor.matmul(wps[:, 0:512], wz[:, 0:128], wz[:, 0:512])

            # q/k for both attns at partitions 0:8; attn an at cols an*N
            q_sb = big.tile([P, 2 * N], F32R, name="q_sb", tag="q_sb")
            k_sb = big.tile([P, 2 * N], F32R, name="k_sb", tag="k_sb")
            vt1b = big.tile([P, IT * C], F32, name="vt1b", tag="vt1b")
            vt2b = big.tile([P, IT * C], F32, name="vt2b", tag="vt2b")
            out_sb = big.tile([P, N], F32, name="out_sb", tag="out_sb")

            def emit_compute():
                # ---- projection phase per column chunk: q/k then V^T tiles
              # weight-major: each (an, q|k) keeps one lhsT for 5 chunks
              for an in (0, 1):
                  rows = slice(0, C) if an == 0 else slice(C, P)
                  for wi, (wsel, dst, bias) in enumerate(
                          ((wq, q_sb, qkbias[an][0]), (wk, k_sb, qkbias[an][1]))):
                      ws = wsel[rows, :]
                      for ci, (off, w) in enumerate(CHUNKS):
                          pq = psA.tile([P, 512], F32,
                                        name=f"pp{an}_{wi}_{ci}", tag="ps")
                          nc.tensor.matmul(pq[0:8, :w], ws,
                                           x_sb[rows, off:off + w])
                          nc.vector.tensor_scalar(
                              out=dst[0:8, an * N + off:an * N + off + w],
                              in0=pq[0:8, :w],
                              scalar1=bias[0:8, :], scalar2=None, op0=ALU.add)
              # V^T tiles (lhsT changes per tile; inherent)
              for t in range(IT):
                  pv1 = psA.tile([P, 512], F32, name=f"pv1_{t}", tag="ps")
                  nc.tensor.matmul(
                      pv1[:, 0:C], x_sb[0:C, t * P:(t + 1) * P], wv[0:C, :])
                  nc.vector.tensor_tensor(
                      out=vt1b[:, t * C:(t + 1) * C], in0=pv1[:, 0:C], in1=bvb[0],
                      op=ALU.add)
                  pv2 = psA.tile([P, 512], F32, name=f"pv2_{t}", tag="ps")
                  nc.tensor.matmul(
                      pv2[:, 0:C], x_sb[C:P, t * P:(t + 1) * P], wv[C:P, :])
                  nc.vector.tensor_tensor(
                      out=vt2b[:, t * C:(t + 1) * C], in0=pv2[:, 0:C], in1=bvb[1],
                      op=ALU.add)

              # ---- main loop over i-tiles
              psum_o = psO.tile([P, N], F32, name="psum_o", tag="pso")

              def emit_omms(t, e1t, e2t, vts):
                  st, sp = (t == 0), (t == IT - 1)
                  for (off, w) in CHUNKS:
                      # o1 (partitions 0:64) <- vt1s^T @ E2 ; o2 <- vt2s^T @ E1
                      nc.tensor.matmul(
                          psum_o[0:C, off:off + w], vts[:, 0:C],
                          e2t[:, off:off + w], start=st, stop=sp)
                      nc.tensor.matmul(
                          psum_o[C:P, off:off + w], vts[:, C:P],
                          e1t[:, off:off + w], start=st, stop=sp)

              # o-mm emission counts after each of the 10 (an, chunk) A-mm
              # positions: keep PE just ahead of ACT, never a block of o-mms.
              O_COUNTS = [0, 0, 2, 2, 2, 2, 2, 0, 0, 0]

              def omm_list(t, e1t, e2t, vts):
                  # lhsT-major: all o1 chunks (one stationary operand), then
                  # all o2 chunks -> 2 weight loads per i-tile instead of 10
                  st, sp = (t == 0), (t == IT - 1)
                  mms = []
                  for (off, w) in CHUNKS:
                      mms.append((psum_o[0:C, off:off + w], vts[:, 0:C],
                                  e2t[:, off:off + w], st, sp))
                  for (off, w) in CHUNKS:
                      mms.append((psum_o[C:P, off:off + w], vts[:, C:P],
                                  e1t[:, off:off + w], st, sp))
                  return mms

              prev = None
              for t in range(IT):
                  pending = omm_list(*prev) if prev is not None else []
                  pi = 0
                  ets = []
                  sps = []
                  for an in (0, 1):
                      qs = slice(an * N + t * P, an * N + (t + 1) * P)
                      et = epool.tile([P, N], BF16, name=f"e{an}_{t}", tag=f"e{an}")
                      sp = small.tile([P, 8], F32, name=f"sp{an}_{t}", tag=f"sp{an}")
                      for ci, (off, w) in enumerate(CHUNKS):
                          pa = psA.tile([P, 512], F32, name=f"pa{an}_{t}_{ci}", tag="ps")
                          nc.tensor.matmul(
                              pa[:, :w], q_sb[0:8, qs],
                              k_sb[0:8, an * N + off:an * N + off + w])
                          nc.scalar.activation(
                              out=et[:, off:off + w], in_=pa[:, :w], func=AF.Exp,
                              accum_out=sp[:, ci:ci + 1])
                          for _ in range(O_COUNTS[an * NCHUNK + ci]):
                              if pending:
                                  o, l, rr_, st_, sp_ = pending.pop(0)
                                  nc.tensor.matmul(o, l, rr_, start=st_, stop=sp_)
                      ets.append(et)
                      sps.append(sp)
                      # row stats as soon as this row's exps are emitted
                      s = small.tile([P, 1], F32, name=f"s{an}_{t}", tag=f"s{an}")
                      nc.vector.tensor_reduce(
                          s, sp[:, 0:NCHUNK], axis=AX.X, op=ALU.add)
                      rr = small.tile([P, 1], F32, name=f"r{an}_{t}", tag=f"r{an}")
                      nc.vector.reciprocal(rr, s)
                      sps.append(rr)
                  for o, l, rr_, st_, sp_ in pending:
                      nc.tensor.matmul(o, l, rr_, start=st_, stop=sp_)
                  r1_, r2_ = sps[1], sps[3]
                  # vts cols 0:64 = vt1b*(1/s2)*gamma ; 64:128 = vt2b*(1/s1)*beta
                  vts = small.tile([P, P], BF16, name=f"vts_{t}", tag="vts")
                  nc.vector.tensor_scalar(
                      out=vts[:, 0:C], in0=vt1b[:, t * C:(t + 1) * C],
                      scalar1=r2_, scalar2=gamma, op0=ALU.mult, op1=ALU.mult)
                  nc.vector.tensor_scalar(
                      out=vts[:, C:P], in0=vt2b[:, t * C:(t + 1) * C],
                      scalar1=r1_, scalar2=beta, op0=ALU.mult, op1=ALU.mult)
                  prev = (t, ets[0], ets[1], vts)
              for o, l, rr_, st_, sp_ in omm_list(*prev):
                  nc.tensor.matmul(o, l, rr_, start=st_, stop=sp_)

              # ---- final: out = psum_o + x  (residual), chunked store
              for ci, (off, w) in enumerate(CHUNKS):
                  nc.vector.tensor_tensor(
                      out=out_sb[:, off:off + w], in0=psum_o[:, off:off + w],
                      in1=x_sb[:, off:off + w].bitcast(F32), op=ALU.add)
                  nc.sync.dma_start(
                      out=out_d.ap()[:, off:off + w], in_=out_sb[:, off:off + w])

            if repeats == 1:
                emit_compute()
            else:
                from concourse import mybir as _mb
                with tc.For_i(0, repeats, 1,
                              hint_engines=(_mb.EngineType.PE,
                                            _mb.EngineType.Activation,
                                            _mb.EngineType.DVE)):
                    emit_compute()

    nc.compile()
    return nc


def _get_nc(repeats=1):
    key = f"nc{repeats}"
    if key not in _CACHE:
        _CACHE[key] = _build(repeats)
    return _CACHE[key]


def _make_in_maps(x1, x2, Wqk1, bqk1, Wqk2, bqk2, Wv1, bv1, Wv2, bv2, gamma, beta):
    f = np.float32
    consts = np.zeros((P, 214), dtype=f)
    consts[0:C, 0:8] = np.asarray(Wqk1, f)[0:CR, :].T
    consts[C:P, 0:8] = np.asarray(Wqk2, f)[0:CR, :].T
    consts[0:C, 8:16] = np.asarray(Wqk1, f)[CR:2 * CR, :].T
    consts[C:P, 8:16] = np.asarray(Wqk2, f)[CR:2 * CR, :].T
    consts[0:C, 16:80] = np.asarray(Wv1, f).T
    consts[C:P, 16:80] = np.asarray(Wv2, f).T
    consts[0:CR, 80] = np.asarray(bqk1, f)[0:CR]
    consts[0:CR, 81] = np.asarray(bqk1, f)[CR:2 * CR]
    consts[0:CR, 82] = np.asarray(bqk2, f)[0:CR]
    consts[0:CR, 83] = np.asarray(bqk2, f)[CR:2 * CR]
    consts[:, 84:148] = np.asarray(bv1, f)[None, :]
    consts[:, 148:212] = np.asarray(bv2, f)[None, :]
    consts[:, 212] = np.float32(np.asarray(gamma, f).reshape(-1)[0])
    consts[:, 213] = np.float32(np.asarray(beta, f).reshape(-1)[0])

    x1 = np.ascontiguousarray(np.asarray(x1, f).reshape(B, C, N))
    x2 = np.ascontiguousarray(np.asarray(x2, f).reshape(B, C, N))
    return [
        {"x1": np.ascontiguousarray(x1[i]), "x2": np.ascontiguousarray(x2[i]),
         "consts": consts}
        for i in range(B)
    ]


def _run(in_maps, repeats=1, **kwargs):
    from concourse.bass_utils import run_bass_kernel_spmd
    nc = _get_nc(repeats)
    return run_bass_kernel_spmd(nc, in_maps, core_ids=list(range(B)), **kwargs)


def kernel(x1, x2, Wqk1, bqk1, Wqk2, bqk2, Wv1, bv1, Wv2, bv2, gamma, beta):
    in_maps = _make_in_maps(x1, x2, Wqk1, bqk1, Wqk2, bqk2, Wv1, bv1, Wv2, bv2,
                            gamma, beta)
    res = _run(in_maps)
    o1 = np.empty((B, C, H, W), dtype=np.float32)
    o2 = np.empty((B, C, H, W), dtype=np.float32)
    for i in range(B):
        full = res.results[i]["out"]
        o1[i] = full[0:C, :].reshape(C, H, W)
        o2[i] = full[C:P, :].reshape(C, H, W)
    return o1, o2



# revision 2
# speedup vs baseline: 1.0464x; 1.0464x over previous
"""CABlock (cross-attention block) Trainium2 Bass kernel, v2.

Problem: b=8, c=64, h=w=48 (n=2304), CR=8.
  qk_i = Wqk_i @ x_i + bqk_i  (q = first 8 rows, k = last 8)
  attn_i = softmax_j(q_i^T k_i)            [n, n]
  o1 = (Wv1@x1 + bv1) @ attn2 * gamma + x1
  o2 = (Wv2@x2 + bv2) @ attn1 * beta  + x2

Sharding: data-parallel over batch, 1 batch element per NeuronCore (8 cores).

v2 layout (ACT-bound redesign; baseline was 180 chunked 512-wide exps paying
a ~370ns fixed cost each = 136us ACT busy):
  - logits for each attention row (t, an) [128, 2304] staged f32 in two
    ping-pong PSUM slots of [128, 1152] (3 banks each); ONE exp instruction
    per half (1152 wide) with accum_out giving the half row-sum.  36 rows x
    2 instrs -> ~96us ACT.
  - q/k in bf16 (fp32r pays 4x at out-free-dim < 256; bf16 is full rate at
    any N).  Logit chunk MMs per half: (512, 512, 128).
  - o1/o2 accumulated in SBUF f32 (`acc`) via DVE flushes; o-matmuls run
    chunk-major over 2-tile groups into 2 rotating single-bank PSUM chunks
    (banks 6-7).  PSUM total = 2*3 + 2 = 8 banks.
  - projections (k1 first, then q1 c0) ordered so the first exp starts ~4us
    in (baseline stalled ACT 26us waiting for the whole projection phase).
  - tiles 16/17 are single-tile o-groups so the post-last-exp tail is short.
"""

import numpy as np

C = 64
CR = 8
H = W = 48
N = H * W            # 2304
B = 8
P = 128
IT = N // P          # 18 i-tiles
CHUNKS = [(0, 512), (512, 512), (1024, 512), (1536, 512), (2048, 256)]
HALF = 1152
HCH = [(0, 512), (512, 512), (1024, 128)]   # chunks within a half
GROUPS = [(0, 1), (2, 3), (4, 5), (6, 7), (8, 9), (10, 11), (12, 13),
          (14, 15), (16,), (17,)]
GROUPS_1 = [(t,) for t in range(IT)]

_CACHE = {}


def _build(repeats=1, groups=None, qk_f32=False):
    import concourse.bacc as bacc
    import concourse.tile as tile
    from concourse import mybir

    F32 = mybir.dt.float32
    F32R = mybir.dt.float32r
    BF16 = mybir.dt.bfloat16
    AF = mybir.ActivationFunctionType
    ALU = mybir.AluOpType
    GRPS = GROUPS if groups is None else groups

    nc = bacc.Bacc("TRN2", target_bir_lowering=False, debug=False, num_devices=8)

    x1_d = nc.dram_tensor("x1", [C, N], F32R, kind="ExternalInput")
    x2_d = nc.dram_tensor("x2", [C, N], F32R, kind="ExternalInput")
    # consts columns: 0:8 wqT, 8:16 wkT, 16:80 wvT, 80 q1bias, 81 k1bias,
    # 82 q2bias, 83 k2bias (rows 0:8), 84:148 bv1 bcast, 148:212 bv2 bcast,
    # 212 gamma, 213 beta
    cst_d = nc.dram_tensor("consts", [P, 214], F32R, kind="ExternalInput")
    out_d = nc.dram_tensor("out", [P, N], F32, kind="ExternalOutput")

    with tile.TileContext(nc) as tc:
        with (
            tc.tile_pool(name="big", bufs=1) as big,
            tc.tile_pool(name="epool", bufs=8) as epool,
            tc.tile_pool(name="vtsp", bufs=4) as vtsp,
            tc.tile_pool(name="small", bufs=6) as small,
            tc.tile_pool(name="rows", bufs=2, space="PSUM") as rows,
            tc.tile_pool(name="po", bufs=2, space="PSUM") as po,
        ):
            # ---- early ACT table warm (loads exp tables during DMA wait)
            warm = big.tile([P, 1], F32, name="warm", tag="warm")
            warm2 = big.tile([P, 1], F32, name="warm2", tag="warm2")
            nc.vector.memset(warm, 0.0)
            nc.scalar.activation(out=warm2, in_=warm, func=AF.Exp)

            # ---- constant + input DMAs (x in column chunks so the k1
            # projection can start as soon as the first chunk lands)
            cst = big.tile([P, 214], F32R, name="cst", tag="cst")
            nc.sync.dma_start(out=cst, in_=cst_d.ap())
            x_sb = big.tile([P, N], F32R, name="x_sb", tag="x_sb")
            for off, w in CHUNKS:
                nc.sync.dma_start(out=x_sb[0:C, off:off + w],
                                  in_=x1_d.ap()[:, off:off + w])
            for off, w in CHUNKS:
                nc.sync.dma_start(out=x_sb[C:P, off:off + w],
                                  in_=x2_d.ap()[:, off:off + w])

            wq = cst[:, 0:8]
            wk = cst[:, 8:16]
            wv = cst[:, 16:80]
            qkbias = [[cst[:, 80:81].bitcast(F32), cst[:, 81:82].bitcast(F32)],
                      [cst[:, 82:83].bitcast(F32), cst[:, 83:84].bitcast(F32)]]
            bvb = [cst[:, 84:148].bitcast(F32), cst[:, 148:212].bitcast(F32)]
            gamma = cst[:, 212:213].bitcast(F32)
            beta = cst[:, 213:214].bitcast(F32)

            # ---- PE HAM warm-up kick (short; real MMs start ~0.5us in)
            wz = big.tile([P, 512], BF16, name="wz", tag="wz")
            nc.vector.memset(wz, 0.0)
            for _wi in range(2):
                wps = po.tile([P, 512], F32, name=f"wps{_wi}", tag="po")
                nc.tensor.matmul(wps[:, 0:512], wz[:, 0:128], wz[:, 0:512])

            # q/k (biased) in bf16; attn an at cols an*N
            QKDT = F32R if qk_f32 else BF16
            q_sb = big.tile([P, 2 * N], QKDT, name="q_sb", tag="q_sb")
            k_sb = big.tile([P, 2 * N], QKDT, name="k_sb", tag="k_sb")
            vt1b = big.tile([P, IT * C], F32, name="vt1b", tag="vt1b")
            vt2b = big.tile([P, IT * C], F32, name="vt2b", tag="vt2b")
            acc = big.tile([P, N], F32, name="acc", tag="acc")
            out_sb = big.tile([P, N], F32, name="out_sb", tag="out_sb")

            def emit_compute():
                # -------- projection helpers
                def proj_qk(an, wi, ci):
                    # wi: 0=q, 1=k; writes (biased, bf16) into q_sb/k_sb
                    rowsl = slice(0, C) if an == 0 else slice(C, P)
                    wsel = wq if wi == 0 else wk
                    dst = q_sb if wi == 0 else k_sb
                    off, w = CHUNKS[ci]
                    pq = po.tile([P, 512], F32, name=f"pp{an}_{wi}_{ci}", tag="po")
                    nc.tensor.matmul(pq[0:8, :w], wsel[rowsl, :],
                                     x_sb[rowsl, off:off + w])
                    nc.vector.tensor_scalar(
                        out=dst[0:8, an * N + off:an * N + off + w],
                        in0=pq[0:8, :w],
                        scalar1=qkbias[an][wi][0:8, :], scalar2=None, op0=ALU.add)

                def proj_v(t):
                    pv1 = po.tile([P, 512], F32, name=f"pv1_{t}", tag="po")
                    nc.tensor.matmul(
                        pv1[:, 0:C], x_sb[0:C, t * P:(t + 1) * P], wv[0:C, :])
                    nc.vector.tensor_tensor(
                        out=vt1b[:, t * C:(t + 1) * C], in0=pv1[:, 0:C],
                        in1=bvb[0], op=ALU.add)
                    pv2 = po.tile([P, 512], F32, name=f"pv2_{t}", tag="po")
                    nc.tensor.matmul(
                        pv2[:, 0:C], x_sb[C:P, t * P:(t + 1) * P], wv[C:P, :])
                    nc.vector.tensor_tensor(
                        out=vt2b[:, t * C:(t + 1) * C], in0=pv2[:, 0:C],
                        in1=bvb[1], op=ALU.add)

                # -------- per-row state
                etile = {}    # (t, an) -> E bf16 [P, N]
                rtile = {}    # (t, an) -> 1/rowsum [P, 1]
                vts = {}      # t -> packed scaled V^T [P, P] bf16

                def emit_row(t, an):
                    qs = slice(an * N + t * P, an * N + (t + 1) * P)
                    et = epool.tile([P, N], BF16, name=f"e{an}_{t}", tag="e")
                    sp = small.tile([P, 2], F32, name=f"sp{an}_{t}", tag="sp")
                    for h in range(2):
                        slot = rows.tile([P, HALF], F32, name=f"sl{an}_{t}_{h}",
                                         tag="row")
                        hb = h * HALF
                        for (off, w) in HCH:
                            nc.tensor.matmul(
                                slot[:, off:off + w], q_sb[0:8, qs],
                                k_sb[0:8, an * N + hb + off:an * N + hb + off + w])
                        nc.scalar.activation(
                            out=et[:, hb:hb + HALF], in_=slot, func=AF.Exp,
                            accum_out=sp[:, h:h + 1])
                    s = small.tile([P, 1], F32, name=f"s{an}_{t}", tag="s")
                    nc.vector.tensor_tensor(
                        out=s, in0=sp[:, 0:1], in1=sp[:, 1:2], op=ALU.add)
                    rr = small.tile([P, 1], F32, name=f"r{an}_{t}", tag="r")
                    nc.vector.reciprocal(rr, s)
                    etile[(t, an)] = et
                    rtile[(t, an)] = rr

                def emit_vts(t):
                    v = vtsp.tile([P, P], BF16, name=f"vts_{t}", tag="vts")
                    nc.vector.tensor_scalar(
                        out=v[:, 0:C], in0=vt1b[:, t * C:(t + 1) * C],
                        scalar1=rtile[(t, 1)], scalar2=gamma,
                        op0=ALU.mult, op1=ALU.mult)
                    nc.vector.tensor_scalar(
                        out=v[:, C:P], in0=vt2b[:, t * C:(t + 1) * C],
                        scalar1=rtile[(t, 0)], scalar2=beta,
                        op0=ALU.mult, op1=ALU.mult)
                    vts[t] = v

                last_group = len(GRPS) - 1

                def emit_ochunk(gi, ci):
                    # one 512-col (or 256) output chunk accumulated over the
                    # group's tiles in a rotating single-bank PSUM tile, then
                    # DVE-flushed into the SBUF accumulator.
                    G = GRPS[gi]
                    off, w = CHUNKS[ci]
                    p = po.tile([P, 512], F32, name=f"oc{gi}_{ci}", tag="po")
                    for idx, t in enumerate(G):
                        st = (idx == 0)
                        sp_ = (idx == len(G) - 1)
                        # o1/o2 run independent accumulation groups on
                        # disjoint partition halves of the same bank; the
                        # interp's zero-region check is whole-bank (the
                        # baseline kernel ran this pattern on HW correctly).
                        nc.tensor.matmul(p[0:C, :w], vts[t][:, 0:C],
                                         etile[(t, 1)][:, off:off + w],
                                         start=st, stop=sp_,
                                         skip_group_check=True)
                        nc.tensor.matmul(p[C:P, :w], vts[t][:, C:P],
                                         etile[(t, 0)][:, off:off + w],
                                         start=st, stop=sp_,
                                         skip_group_check=True)
                    if gi == 0:
                        nc.vector.tensor_copy(out=acc[:, off:off + w],
                                              in_=p[:, :w])
                    elif gi == last_group:
                        # out_sb already holds acc+x (preacc below); fold the
                        # last tile group in-place and ship it.
                        nc.vector.tensor_tensor(
                            out=out_sb[:, off:off + w], in0=p[:, :w],
                            in1=out_sb[:, off:off + w], op=ALU.add)
                        nc.sync.dma_start(out=out_d.ap()[:, off:off + w],
                                          in_=out_sb[:, off:off + w])
                    else:
                        nc.vector.tensor_tensor(
                            out=acc[:, off:off + w], in0=p[:, :w],
                            in1=acc[:, off:off + w], op=ALU.add)
                    if gi == last_group - 1:
                        # precompute residual+accumulated-o for this chunk so
                        # the post-last-exp tail is one DVE op per chunk.
                        nc.vector.tensor_tensor(
                            out=out_sb[:, off:off + w], in0=acc[:, off:off + w],
                            in1=x_sb[:, off:off + w].bitcast(F32), op=ALU.add)

                # -------- head: k1 first, then first two an0 rows (only
                # need k1+q1c0); k2/q2 projections hide under their exps.
                # Row order is group-local [(t0,0),(t1,0),(t0,1),(t1,1)] so
                # an1 projections never gate the ACT stream.
                rows_sched = []
                for G in GRPS:
                    for an in (0, 1):
                        for t in G:
                            rows_sched.append((t, an))
                # global row index after which group gi is fully emitted
                ready_at = {}
                for gi, G in enumerate(GRPS):
                    ready_at[max(rows_sched.index((t, 1)) for t in G)] = gi

                for ci in range(5):
                    proj_qk(0, 1, ci)          # k1
                proj_qk(0, 0, 0)               # q1 c0
                n_head = 2 if len(GRPS[0]) == 2 else 1
                for (t, an) in rows_sched[:n_head]:
                    emit_row(t, an)            # (0,0) [, (1,0)]
                proj_v(0)
                if len(GRPS[0]) == 2:
                    proj_v(1)
                for ci in range(5):
                    proj_qk(1, 1, ci)          # k2
                proj_qk(1, 0, 0)               # q2 c0
                pending = []
                vleft = list(range(2 if len(GRPS[0]) == 2 else 1, IT))
                for r in range(n_head, len(rows_sched)):
                    t, an = rows_sched[r]
                    emit_row(t, an)
                    if r == n_head:            # right after first an1 row
                        for ci in range(1, 5):
                            proj_qk(0, 0, ci)  # q1 c1..c4
                        for ci in range(1, 5):
                            proj_qk(1, 0, ci)  # q2 c1..c4
                    if vleft and r < 18:
                        proj_v(vleft.pop(0))
                    if an == 1:
                        emit_vts(t)
                    if (r - 1) in ready_at:
                        gi = ready_at[r - 1]
                        pending.extend((gi, ci) for ci in range(5))
                    for _ in range(1 if r <= 17 else 2):
                        if pending:
                            emit_ochunk(*pending.pop(0))
                # tail: remaining chunks (last groups)
                pending.extend((ready_at[len(rows_sched) - 1], ci)
                               for ci in range(5))
                for gi, ci in pending:
                    emit_ochunk(gi, ci)

            if repeats == 1:
                emit_compute()
            else:
                from concourse import mybir as _mb
                with tc.For_i(0, repeats, 1,
                              hint_engines=(_mb.EngineType.PE,
                                            _mb.EngineType.Activation,
                                            _mb.EngineType.DVE)):
                    emit_compute()

    nc.compile()
    return nc


def _get_nc(repeats=1):
    key = f"nc{repeats}"
    if key not in _CACHE:
        _CACHE[key] = _build(repeats)
    return _CACHE[key]


def _make_in_maps(x1, x2, Wqk1, bqk1, Wqk2, bqk2, Wv1, bv1, Wv2, bv2, gamma, beta):
    f = np.float32
    consts = np.zeros((P, 214), dtype=f)
    consts[0:C, 0:8] = np.asarray(Wqk1, f)[0:CR, :].T
    consts[C:P, 0:8] = np.asarray(Wqk2, f)[0:CR, :].T
    consts[0:C, 8:16] = np.asarray(Wqk1, f)[CR:2 * CR, :].T
    consts[C:P, 8:16] = np.asarray(Wqk2, f)[CR:2 * CR, :].T
    consts[0:C, 16:80] = np.asarray(Wv1, f).T
    consts[C:P, 16:80] = np.asarray(Wv2, f).T
    consts[0:CR, 80] = np.asarray(bqk1, f)[0:CR]
    consts[0:CR, 81] = np.asarray(bqk1, f)[CR:2 * CR]
    consts[0:CR, 82] = np.asarray(bqk2, f)[0:CR]
    consts[0:CR, 83] = np.asarray(bqk2, f)[CR:2 * CR]
    consts[:, 84:148] = np.asarray(bv1, f)[None, :]
    consts[:, 148:212] = np.asarray(bv2, f)[None, :]
    consts[:, 212] = np.float32(np.asarray(gamma, f).reshape(-1)[0])
    consts[:, 213] = np.float32(np.asarray(beta, f).reshape(-1)[0])

    x1 = np.ascontiguousarray(np.asarray(x1, f).reshape(B, C, N))
    x2 = np.ascontiguousarray(np.asarray(x2, f).reshape(B, C, N))
    return [
        {"x1": np.ascontiguousarray(x1[i]), "x2": np.ascontiguousarray(x2[i]),
         "consts": consts}
        for i in range(B)
    ]


def _run(in_maps, repeats=1, **kwargs):
    from concourse.bass_utils import run_bass_kernel_spmd
    nc = _get_nc(repeats)
    return run_bass_kernel_spmd(nc, in_maps, core_ids=list(range(B)), **kwargs)


def kernel(x1, x2, Wqk1, bqk1, Wqk2, bqk2, Wv1, bv1, Wv2, bv2, gamma, beta):
    in_maps = _make_in_maps(x1, x2, Wqk1, bqk1, Wqk2, bqk2, Wv1, bv1, Wv2, bv2,
                            gamma, beta)
    res = _run(in_maps)
    o1 = np.empty((B, C, H, W), dtype=np.float32)
    o2 = np.empty((B, C, H, W), dtype=np.float32)
    for i in range(B):
        full = res.results[i]["out"]
        o1[i] = full[0:C, :].reshape(C, H, W)
        o2[i] = full[C:P, :].reshape(C, H, W)
    return o1, o2


# revision 3
# speedup vs baseline: 1.1474x; 1.0966x over previous
"""CABlock (cross-attention block) Trainium2 Bass kernel, v2.

Problem: b=8, c=64, h=w=48 (n=2304), CR=8.
  qk_i = Wqk_i @ x_i + bqk_i  (q = first 8 rows, k = last 8)
  attn_i = softmax_j(q_i^T k_i)            [n, n]
  o1 = (Wv1@x1 + bv1) @ attn2 * gamma + x1
  o2 = (Wv2@x2 + bv2) @ attn1 * beta  + x2

Sharding: data-parallel over batch, 1 batch element per NeuronCore (8 cores).

v2 layout (ACT-bound redesign; baseline was 180 chunked 512-wide exps paying
a ~370ns fixed cost each = 136us ACT busy):
  - logits for each attention row (t, an) [128, 2304] staged f32 in two
    ping-pong PSUM slots of [128, 1152] (3 banks each); ONE exp instruction
    per half (1152 wide) with accum_out giving the half row-sum.  36 rows x
    2 instrs -> ~96us ACT.
  - q/k in bf16 (fp32r pays 4x at out-free-dim < 256; bf16 is full rate at
    any N).  Logit chunk MMs per half: (512, 512, 128).
  - o1/o2 accumulated in SBUF f32 (`acc`) via DVE flushes; o-matmuls run
    chunk-major over 2-tile groups into 2 rotating single-bank PSUM chunks
    (banks 6-7).  PSUM total = 2*3 + 2 = 8 banks.
  - projections (k1 first, then q1 c0) ordered so the first exp starts ~4us
    in (baseline stalled ACT 26us waiting for the whole projection phase).
  - tiles 16/17 are single-tile o-groups so the post-last-exp tail is short.
"""

import numpy as np

C = 64
CR = 8
H = W = 48
N = H * W            # 2304
B = 8
P = 128
IT = N // P          # 18 i-tiles
CHUNKS = [(0, 512), (512, 512), (1024, 512), (1536, 512), (2048, 256)]
HALF = 1152
HCH = [(0, 512), (512, 512), (1024, 128)]   # chunks within a half
GROUPS = [(0, 1), (2, 3), (4, 5), (6, 7), (8, 9), (10, 11), (12, 13),
          (14, 15), (16,), (17,)]
GROUPS_1 = [(t,) for t in range(IT)]

_CACHE = {}


def _build(repeats=1, groups=None, qk_f32=False, exp_mode='wide'):
    import concourse.bacc as bacc
    import concourse.tile as tile
    from concourse import mybir

    F32 = mybir.dt.float32
    F32R = mybir.dt.float32r
    BF16 = mybir.dt.bfloat16
    AF = mybir.ActivationFunctionType
    ALU = mybir.AluOpType
    AX = mybir.AxisListType
    GRPS = GROUPS if groups is None else groups

    nc = bacc.Bacc("TRN2", target_bir_lowering=False, debug=False, num_devices=8)

    x1_d = nc.dram_tensor("x1", [C, N], F32R, kind="ExternalInput")
    x2_d = nc.dram_tensor("x2", [C, N], F32R, kind="ExternalInput")
    # consts columns: 0:8 wqT, 8:16 wkT, 16:80 wvT, 80 q1bias, 81 k1bias,
    # 82 q2bias, 83 k2bias (rows 0:8), 84:148 bv1 bcast, 148:212 bv2 bcast,
    # 212 gamma, 213 beta
    cst_d = nc.dram_tensor("consts", [P, 214], F32R, kind="ExternalInput")
    out_d = nc.dram_tensor("out", [P, N], F32, kind="ExternalOutput")

    with tile.TileContext(nc) as tc:
        with (
            tc.tile_pool(name="big", bufs=1) as big,
            tc.tile_pool(name="epool", bufs=8) as epool,
            tc.tile_pool(name="vtsp", bufs=4) as vtsp,
            tc.tile_pool(name="small", bufs=6) as small,
            tc.tile_pool(name="rows", bufs=2, space="PSUM") as rows,
            tc.tile_pool(name="po", bufs=2, space="PSUM") as po,
        ):
            # ---- early ACT table warm (loads exp tables during DMA wait)
            warm = big.tile([P, 1], F32, name="warm", tag="warm")
            warm2 = big.tile([P, 1], F32, name="warm2", tag="warm2")
            nc.vector.memset(warm, 0.0)
            nc.scalar.activation(out=warm2, in_=warm, func=AF.Exp)

            # ---- constant + input DMAs (x in column chunks so the k1
            # projection can start as soon as the first chunk lands)
            cst = big.tile([P, 214], F32R, name="cst", tag="cst")
            nc.sync.dma_start(out=cst, in_=cst_d.ap())
            x_sb = big.tile([P, N], F32R, name="x_sb", tag="x_sb")
            for off, w in ((0, 1024), (1024, 1280)):
                nc.sync.dma_start(out=x_sb[0:C, off:off + w],
                                  in_=x1_d.ap()[:, off:off + w])
            for off, w in ((0, 1024), (1024, 1280)):
                nc.sync.dma_start(out=x_sb[C:P, off:off + w],
                                  in_=x2_d.ap()[:, off:off + w])

            wq = cst[:, 0:8]
            wk = cst[:, 8:16]
            wv = cst[:, 16:80]
            qkbias = [[cst[:, 80:81].bitcast(F32), cst[:, 81:82].bitcast(F32)],
                      [cst[:, 82:83].bitcast(F32), cst[:, 83:84].bitcast(F32)]]
            bvb = [cst[:, 84:148].bitcast(F32), cst[:, 148:212].bitcast(F32)]
            gamma = cst[:, 212:213].bitcast(F32)
            beta = cst[:, 213:214].bitcast(F32)

            # ---- PE HAM warm-up kick (short; real MMs start ~0.5us in)
            wz = big.tile([P, 512], BF16, name="wz", tag="wz")
            nc.vector.memset(wz, 0.0)
            for _wi in range(2):
                wps = po.tile([P, 512], F32, name=f"wps{_wi}", tag="po")
                nc.tensor.matmul(wps[:, 0:512], wz[:, 0:128], wz[:, 0:512])

            # q/k (biased) in bf16; attn an at cols an*N
            QKDT = F32R if qk_f32 else BF16
            q_sb = big.tile([P, 2 * N], QKDT, name="q_sb", tag="q_sb")
            k_sb = big.tile([P, 2 * N], QKDT, name="k_sb", tag="k_sb")
            vt1b = big.tile([P, IT * C], F32, name="vt1b", tag="vt1b")
            vt2b = big.tile([P, IT * C], F32, name="vt2b", tag="vt2b")
            acc = big.tile([P, N], F32, name="acc", tag="acc")
            out_sb = big.tile([P, N], F32, name="out_sb", tag="out_sb")

            def emit_compute():
                # -------- projection helpers
                def proj_qk(an, wi, ci):
                    # wi: 0=q, 1=k; writes (biased, bf16) into q_sb/k_sb
                    rowsl = slice(0, C) if an == 0 else slice(C, P)
                    wsel = wq if wi == 0 else wk
                    dst = q_sb if wi == 0 else k_sb
                    off, w = CHUNKS[ci]
                    pq = po.tile([P, 512], F32, name=f"pp{an}_{wi}_{ci}", tag="po")
                    nc.tensor.matmul(pq[0:8, :w], wsel[rowsl, :],
                                     x_sb[rowsl, off:off + w])
                    nc.vector.tensor_scalar(
                        out=dst[0:8, an * N + off:an * N + off + w],
                        in0=pq[0:8, :w],
                        scalar1=qkbias[an][wi][0:8, :], scalar2=None, op0=ALU.add)

                def proj_v(t):
                    pv1 = po.tile([P, 512], F32, name=f"pv1_{t}", tag="po")
                    nc.tensor.matmul(
                        pv1[:, 0:C], x_sb[0:C, t * P:(t + 1) * P], wv[0:C, :])
                    nc.vector.tensor_tensor(
                        out=vt1b[:, t * C:(t + 1) * C], in0=pv1[:, 0:C],
                        in1=bvb[0], op=ALU.add)
                    pv2 = po.tile([P, 512], F32, name=f"pv2_{t}", tag="po")
                    nc.tensor.matmul(
                        pv2[:, 0:C], x_sb[C:P, t * P:(t + 1) * P], wv[C:P, :])
                    nc.vector.tensor_tensor(
                        out=vt2b[:, t * C:(t + 1) * C], in0=pv2[:, 0:C],
                        in1=bvb[1], op=ALU.add)

                # -------- per-row state
                etile = {}    # (t, an) -> E bf16 [P, N]
                rtile = {}    # (t, an) -> 1/rowsum [P, 1]
                vts = {}      # t -> packed scaled V^T [P, P] bf16

                def emit_row(t, an, between=None):
                    qs = slice(an * N + t * P, an * N + (t + 1) * P)
                    et = epool.tile([P, N], BF16, name=f"e{an}_{t}", tag="e")
                    nsp = 2 if exp_mode == 'wide' else 6
                    sp = small.tile([P, nsp], F32, name=f"sp{an}_{t}", tag="sp")
                    for h in range(2):
                        slot = rows.tile([P, HALF], F32, name=f"sl{an}_{t}_{h}",
                                         tag="row")
                        hb = h * HALF
                        for (off, w) in HCH:
                            nc.tensor.matmul(
                                slot[:, off:off + w], q_sb[0:8, qs],
                                k_sb[0:8, an * N + hb + off:an * N + hb + off + w])
                        if exp_mode == 'wide':
                            nc.scalar.activation(
                                out=et[:, hb:hb + HALF], in_=slot, func=AF.Exp,
                                accum_out=sp[:, h:h + 1])
                        else:
                            for ci, (off, w) in enumerate(HCH):
                                nc.scalar.activation(
                                    out=et[:, hb + off:hb + off + w],
                                    in_=slot[:, off:off + w], func=AF.Exp,
                                    accum_out=sp[:, 3 * h + ci:3 * h + ci + 1])
                        if h == 0 and between is not None:
                            between()
                    s = small.tile([P, 1], F32, name=f"s{an}_{t}", tag="s")
                    if exp_mode == 'wide':
                        nc.vector.tensor_tensor(
                            out=s, in0=sp[:, 0:1], in1=sp[:, 1:2], op=ALU.add)
                    else:
                        nc.vector.tensor_reduce(
                            s, sp[:, 0:6], axis=AX.X, op=ALU.add)
                    rr = small.tile([P, 1], F32, name=f"r{an}_{t}", tag="r")
                    nc.vector.reciprocal(rr, s)
                    etile[(t, an)] = et
                    rtile[(t, an)] = rr

                def emit_vts(t):
                    v = vtsp.tile([P, P], BF16, name=f"vts_{t}", tag="vts")
                    nc.vector.tensor_scalar(
                        out=v[:, 0:C], in0=vt1b[:, t * C:(t + 1) * C],
                        scalar1=rtile[(t, 1)], scalar2=gamma,
                        op0=ALU.mult, op1=ALU.mult)
                    nc.vector.tensor_scalar(
                        out=v[:, C:P], in0=vt2b[:, t * C:(t + 1) * C],
                        scalar1=rtile[(t, 0)], scalar2=beta,
                        op0=ALU.mult, op1=ALU.mult)
                    vts[t] = v

                last_group = len(GRPS) - 1

                def emit_ochunk(gi, ci):
                    # one 512-col (or 256) output chunk accumulated over the
                    # group's tiles in a rotating single-bank PSUM tile, then
                    # DVE-flushed into the SBUF accumulator.
                    G = GRPS[gi]
                    off, w = CHUNKS[ci]
                    p = po.tile([P, 512], F32, name=f"oc{gi}_{ci}", tag="po")
                    for idx, t in enumerate(G):
                        st = (idx == 0)
                        sp_ = (idx == len(G) - 1)
                        # o1/o2 run independent accumulation groups on
                        # disjoint partition halves of the same bank; the
                        # interp's zero-region check is whole-bank (the
                        # baseline kernel ran this pattern on HW correctly).
                        nc.tensor.matmul(p[0:C, :w], vts[t][:, 0:C],
                                         etile[(t, 1)][:, off:off + w],
                                         start=st, stop=sp_,
                                         skip_group_check=True)
                        nc.tensor.matmul(p[C:P, :w], vts[t][:, C:P],
                                         etile[(t, 0)][:, off:off + w],
                                         start=st, stop=sp_,
                                         skip_group_check=True)
                    if gi == 0:
                        nc.vector.tensor_copy(out=acc[:, off:off + w],
                                              in_=p[:, :w])
                    elif gi == last_group:
                        # out_sb already holds acc+x (preacc below); fold the
                        # last tile group in-place and ship it.
                        nc.vector.tensor_tensor(
                            out=out_sb[:, off:off + w], in0=p[:, :w],
                            in1=out_sb[:, off:off + w], op=ALU.add)
                        nc.sync.dma_start(out=out_d.ap()[:, off:off + w],
                                          in_=out_sb[:, off:off + w])
                    else:
                        nc.vector.tensor_tensor(
                            out=acc[:, off:off + w], in0=p[:, :w],
                            in1=acc[:, off:off + w], op=ALU.add)
                    if gi == last_group - 1:
                        # precompute residual+accumulated-o for this chunk so
                        # the post-last-exp tail is one DVE op per chunk.
                        nc.vector.tensor_tensor(
                            out=out_sb[:, off:off + w], in0=acc[:, off:off + w],
                            in1=x_sb[:, off:off + w].bitcast(F32), op=ALU.add)

                # -------- head: k1 first, then first two an0 rows (only
                # need k1+q1c0); k2/q2 projections hide under their exps.
                # Row order is group-local [(t0,0),(t1,0),(t0,1),(t1,1)] so
                # an1 projections never gate the ACT stream.
                rows_sched = []
                for G in GRPS:
                    for an in (0, 1):
                        for t in G:
                            rows_sched.append((t, an))
                # global row index after which group gi is fully emitted
                ready_at = {}
                for gi, G in enumerate(GRPS):
                    ready_at[max(rows_sched.index((t, 1)) for t in G)] = gi

                proj_qk(0, 0, 0)               # q1 c0 (heads the copy chain)
                for ci in range(3):
                    proj_qk(0, 1, ci)          # k1 c0..c2 (h0 of row 0)
                n_head = 2 if len(GRPS[0]) == 2 else 1
                emit_row(*rows_sched[0],
                         between=lambda: (proj_qk(0, 1, 3), proj_qk(0, 1, 4)))
                if n_head == 2:
                    emit_row(*rows_sched[1])
                proj_v(0)
                if len(GRPS[0]) == 2:
                    proj_v(1)
                for ci in range(5):
                    proj_qk(1, 1, ci)          # k2
                proj_qk(1, 0, 0)               # q2 c0
                pending = []
                vleft = list(range(2 if len(GRPS[0]) == 2 else 1, IT))
                for r in range(n_head, len(rows_sched)):
                    t, an = rows_sched[r]
                    emit_row(t, an)
                    if r == n_head:            # right after first an1 row
                        for ci in range(1, 5):
                            proj_qk(0, 0, ci)  # q1 c1..c4
                        for ci in range(1, 5):
                            proj_qk(1, 0, ci)  # q2 c1..c4
                    if vleft and r < 18:
                        proj_v(vleft.pop(0))
                    if an == 1:
                        emit_vts(t)
                    if (r - 1) in ready_at:
                        gi = ready_at[r - 1]
                        pending.extend((gi, ci) for ci in range(5))
                    for _ in range(1 if r <= 17 else 2):
                        if pending:
                            emit_ochunk(*pending.pop(0))
                # tail: remaining chunks (last groups)
                pending.extend((ready_at[len(rows_sched) - 1], ci)
                               for ci in range(5))
                for gi, ci in pending:
                    emit_ochunk(gi, ci)

            if repeats == 1:
                emit_compute()
            else:
                from concourse import mybir as _mb
                with tc.For_i(0, repeats, 1,
                              hint_engines=(_mb.EngineType.PE,
                                            _mb.EngineType.Activation,
                                            _mb.EngineType.DVE)):
                    emit_compute()

    nc.compile()
    return nc


def _get_nc(repeats=1):
    key = f"nc{repeats}"
    if key not in _CACHE:
        _CACHE[key] = _build(repeats)
    return _CACHE[key]


def _make_in_maps(x1, x2, Wqk1, bqk1, Wqk2, bqk2, Wv1, bv1, Wv2, bv2, gamma, beta):
    f = np.float32
    consts = np.zeros((P, 214), dtype=f)
    consts[0:C, 0:8] = np.asarray(Wqk1, f)[0:CR, :].T
    consts[C:P, 0:8] = np.asarray(Wqk2, f)[0:CR, :].T
    consts[0:C, 8:16] = np.asarray(Wqk1, f)[CR:2 * CR, :].T
    consts[C:P, 8:16] = np.asarray(Wqk2, f)[CR:2 * CR, :].T
    consts[0:C, 16:80] = np.asarray(Wv1, f).T
    consts[C:P, 16:80] = np.asarray(Wv2, f).T
    consts[0:CR, 80] = np.asarray(bqk1, f)[0:CR]
    consts[0:CR, 81] = np.asarray(bqk1, f)[CR:2 * CR]
    consts[0:CR, 82] = np.asarray(bqk2, f)[0:CR]
    consts[0:CR, 83] = np.asarray(bqk2, f)[CR:2 * CR]
    consts[:, 84:148] = np.asarray(bv1, f)[None, :]
    consts[:, 148:212] = np.asarray(bv2, f)[None, :]
    consts[:, 212] = np.float32(np.asarray(gamma, f).reshape(-1)[0])
    consts[:, 213] = np.float32(np.asarray(beta, f).reshape(-1)[0])

    x1 = np.ascontiguousarray(np.asarray(x1, f).reshape(B, C, N))
    x2 = np.ascontiguousarray(np.asarray(x2, f).reshape(B, C, N))
    return [
        {"x1": np.ascontiguousarray(x1[i]), "x2": np.ascontiguousarray(x2[i]),
         "consts": consts}
        for i in range(B)
    ]


def _run(in_maps, repeats=1, **kwargs):
    from concourse.bass_utils import run_bass_kernel_spmd
    nc = _get_nc(repeats)
    return run_bass_kernel_spmd(nc, in_maps, core_ids=list(range(B)), **kwargs)


def kernel(x1, x2, Wqk1, bqk1, Wqk2, bqk2, Wv1, bv1, Wv2, bv2, gamma, beta):
    in_maps = _make_in_maps(x1, x2, Wqk1, bqk1, Wqk2, bqk2, Wv1, bv1, Wv2, bv2,
                            gamma, beta)
    res = _run(in_maps)
    o1 = np.empty((B, C, H, W), dtype=np.float32)
    o2 = np.empty((B, C, H, W), dtype=np.float32)
    for i in range(B):
        full = res.results[i]["out"]
        o1[i] = full[0:C, :].reshape(C, H, W)
        o2[i] = full[C:P, :].reshape(C, H, W)
    return o1, o2
